# revision 17
# baseline (speedup 1.0000x reference)
"""BiLSTM 2-layer + LayerNorm Trainium2 kernel.

Strategy: data-parallel over batch (8 cores x 8 batch). Each core runs the
full network on its batch shard:
  A: input projections zx1 = x @ W1{f,b}   (bwd written time-reversed)
  B: layer-1 fwd+bwd recurrences, interleaved per step
  C: LayerNorm (folded into projection) + zx2 = LN(h1) @ W2{f,b}
  D: layer-2 recurrences, transposed output written directly

Layouts (per core):
  zx (DRAM, bf16): [d2, uh2, g4, u128, T, b8]; gate order [i, f, o, g~];
    bwd (d=1) plane stored time-reversed, so both recurrences read ascending.
  h1T (DRAM, f32): [d2, uh2, u128, T, b8]; bwd plane in step order (rev time).
  Recurrence state: h.T [u128, kc2, b8] (bf16 for matmul), c [u128, d2, uh2, b8] f32.
  Recurrent GEMM: z.T[(g,uh) tile, (d,b)] += U[kc-chunk, (g,uh)-cols].T @ h.T[kc]
"""
import contextlib
import ctypes
import sys
import types

import ml_dtypes
import numpy as np

import concourse.bass as bass
import concourse.tile as tile
from concourse import mybir
from concourse import bass_utils
from concourse.bass import ds, ts
from concourse.masks import make_identity
from concourse.vector_clock import ScopedClock
from contextlib import ExitStack

# ---------------------------------------------------------------- boot patches
MAXW = 1  # this walrus build allows only 1 sem-wait per instruction


def _patched_drain_and_barrier(self, tick_clock, wait_clock):
    drain_inst = self.nc.sync.drain()
    wait_clock.add_sem_waits(drain_inst.ins, ScopedClock({None: tick_clock.global_clock}))
    si = drain_inst.ins.sync_info
    waits = list(si.on_wait) if si is not None and si.on_wait else []
    if len(waits) > MAXW:
        si.on_wait = waits[:MAXW]
        rest = waits[MAXW:]
        while rest:
            d2 = self.nc.sync.drain()
            d2.ins.sync_info = mybir.SyncInfo(on_wait=rest[:MAXW], on_update=[])
            rest = rest[MAXW:]
    self.nc.all_engine_barrier()
    assert self.sems is not None
    popped = self.nc._tile_sem_poison_stack.pop()
    assert popped is self._sem_poison
    self.nc.clear_and_free_semaphores(list(self.sems.allocated().values()))
    self.nc.all_engine_barrier()


tile.TileContext._drain_and_barrier = _patched_drain_and_barrier


def split_ctrl_waits(nc):
    """Hoist extra sem-waits (>1 per instruction) onto preceding NoOps."""
    n_split = 0
    for f in nc.m.functions:
        for bb in f.blocks:
            new_insts = []
            for inst in bb.instructions:
                si = getattr(inst, "sync_info", None)
                waits = list(si.on_wait) if si is not None and si.on_wait else []
                if len(waits) > MAXW:
                    rest, tail = waits[:-MAXW], waits[-MAXW:]
                    while rest:
                        d = mybir.InstNoOp(
                            name=nc.get_next_instruction_name(),
                            engine=inst.engine,
                            bass_nofuse=True,
                            sync_info=mybir.SyncInfo(on_wait=rest[:MAXW], on_update=[]),
                        )
                        new_insts.append(d)
                        rest = rest[MAXW:]
                    si.on_wait = tail
                    n_split += 1
                new_insts.append(inst)
            bb.instructions[:] = new_insts
    return n_split


# ---------------------------------------------------------------- constants
B, T_FULL, F, U = 64, 1024, 128, 256
NCORES = 8
BS = B // NCORES          # batch per core
LN_EPS = 1e-3
UB = 8                    # recurrence steps per For_i iteration

f32 = mybir.dt.float32
bf16 = mybir.dt.bfloat16
AF = mybir.ActivationFunctionType

# gate order in our layout: [i, f, o, g~]; original keras order [i, f, g, o]
GMAP = [0, 1, 3, 2]  # ours g -> original gate index
ml_bf16 = ml_dtypes.bfloat16


# ---------------------------------------------------------------- program
def build_program(T=T_FULL, dbg=False, split=True):
    nc = bass.Bass("TRN2", target_bir_lowering=False, debug=False)

    x_in = nc.dram_tensor("x_sh", [BS, T, F], f32, kind="ExternalInput").ap()
    # packed weights, host-prepared (see pack_weights):
    w1 = nc.dram_tensor("w1", [2, 4, 2, F, 128], bf16, kind="ExternalInput").ap()
    u1 = nc.dram_tensor("u1", [2, 4, 2, 2, 128, 128], bf16, kind="ExternalInput").ap()
    w2 = nc.dram_tensor("w2", [2, 4, 2, 4, 128, 128], f32, kind="ExternalInput").ap()
    u2 = nc.dram_tensor("u2", [2, 4, 2, 2, 128, 128], bf16, kind="ExternalInput").ap()
    out = nc.dram_tensor("out_sh", [BS, T, 2 * U], f32, kind="ExternalOutput").ap()

    assert T % 64 == 0, "phase A/C block size needs T % 64 == 0"
    NB = T // 64  # number of 512-col (t64 x b8) blocks

    with tile.TileContext(nc) as tc, ExitStack() as octx:
        const = octx.enter_context(tc.tile_pool(name="const", bufs=1))
        dram = octx.enter_context(tc.tile_pool(name="dram", bufs=1, space="DRAM"))
        # PSUM is bump-allocated across the whole context: share pools globally.
        # Budget (8 banks): mmp 4 (tag ps) + tpp 2 (tag tp) + stp 2 (s_sum/s_sq)
        mmp = octx.enter_context(tc.tile_pool(name="ps_mm", bufs=4, space="PSUM"))
        tpp = octx.enter_context(tc.tile_pool(name="ps_tp", bufs=2, space="PSUM"))
        stp = octx.enter_context(tc.tile_pool(name="ps_st", bufs=1, space="PSUM"))

        zx1 = dram.tile([2, 2, 4, 128, T, BS], bf16)
        h1T = dram.tile([2, 2, 128, T, BS], f32)
        zx2 = dram.tile([2, 2, 4, 128, T, BS], bf16)
        o_stg = dram.tile([BS, T, 256], f32)  # bwd half of out (dynamic rev writes)

        ident = const.tile([128, 128], f32)
        make_identity(nc, ident)
        ones_k = const.tile([128, 1], f32)
        nc.vector.memset(ones_k, 1.0)
        ones_m = const.tile([1, 128], f32)
        nc.vector.memset(ones_m, 1.0)
        eps_c = const.tile([1, 1], f32)
        nc.vector.memset(eps_c, LN_EPS)

        # ---------------- phase A: layer-1 projections ----------------
        with ExitStack() as ctx:
            w1sb = const.tile([F, 2, 4, 2, 128], bf16)
            nc.sync.dma_start(w1sb[:], w1.rearrange("d g uh f m -> f d g uh m"))
            stage = ctx.enter_context(tc.tile_pool(name="a_stage", bufs=3))
            xtp = ctx.enter_context(tc.tile_pool(name="a_xt", bufs=2))
            outp = ctx.enter_context(tc.tile_pool(name="a_out", bufs=4))

            for nb in range(NB):
                t0 = nb * 64
                xT = xtp.tile([128, 512], bf16, tag="xT")  # [f, (t64 b8)]
                for j in range(4):
                    xa = stage.tile([128, F], f32, tag="xa")  # [(t16 b8), f]
                    src = bass.AP(
                        tensor=x_in.tensor,
                        offset=x_in.offset + (t0 + j * 16) * F,
                        ap=[[F, 16], [T * F, BS], [1, F]],
                    )
                    nc.sync.dma_start(xa[:], src)
                    tp = tpp.tile([128, 128], f32, tag="tp")
                    nc.tensor.transpose(tp[:], xa[:], ident[:])
                    nc.vector.tensor_copy(xT[:, ts(j, 128)], tp[:])
                for d in range(2):
                    for g in range(4):
                        for uh in range(2):
                            ps = mmp.tile([128, 512], f32, tag="ps")
                            nc.tensor.matmul(ps[:], w1sb[:, d, g, uh, :], xT[:],
                                             start=True, stop=True)
                            ob = outp.tile([128, 512], bf16, tag="ob")
                            eng = nc.vector.tensor_copy if (g % 2) else (
                                lambda o, i: nc.scalar.activation(o, i, AF.Copy))
                            eng(ob[:], ps[:])
                            plane = zx1[d, uh, g]  # [128, T, BS]
                            if d == 0:
                                nc.sync.dma_start(plane[:, t0:t0 + 64, :], ob[:])
                            else:
                                dst = bass.AP(
                                    tensor=plane.tensor,
                                    offset=plane.offset + ((T - 1 - t0) * BS),
                                    ap=[[T * BS, 128], [-BS, 64], [1, BS]],
                                )
                                nc.sync.dma_start(dst, ob[:])

        # ---------------- phase B/D: recurrences ----------------
        def recurrence(layer, zx, u_w, hout, ctx):
            """layer 1: write h1T planes; layer 2: write transposed out."""
            uwsb = const.tile([128, 2, 4, 2, 2, 128], bf16, tag=f"uw{layer}")
            nc.sync.dma_start(uwsb[:], u_w.rearrange("d g uh kc k m -> k d g uh kc m"))

            state = ctx.enter_context(tc.tile_pool(name=f"r{layer}_state", bufs=1))
            zxp = ctx.enter_context(tc.tile_pool(name=f"r{layer}_zx", bufs=2))
            work = ctx.enter_context(tc.tile_pool(name=f"r{layer}_work", bufs=3))
            ringp = ctx.enter_context(tc.tile_pool(name=f"r{layer}_ring", bufs=2))
            osb = ctx.enter_context(tc.tile_pool(name=f"r{layer}_osb", bufs=2))

            # h ping-pong (bf16, matmul rhs layout [u, kc, b]); c state f32
            h_pp = state.tile([128, 2, 2, 2, BS], bf16)  # [u, pp, d, kc, b]
            c_sb = state.tile([128, 2, 2, BS], f32)      # [u, d, uh, b]
            nc.vector.memset(h_pp[:], 0.0)
            nc.vector.memset(c_sb[:], 0.0)

            with tc.For_i(0, T, UB) as s0:
                zx_sb = zxp.tile([128, 2, 2, 4, UB, BS], bf16, tag="zx_sb")  # [u, uh, d, g, s, b]
                for d in range(2):
                    for uh in range(2):
                        nc.scalar.dma_start(
                            zx_sb[:, uh, d, :, :, :],
                            zx[d, uh, :, :, ds(s0, UB), :].rearrange("g u s b -> u g s b"))
                ring = ringp.tile([128, 2, 2, UB, BS], f32, tag="ring")  # [u, d, uh, s, b]
                for k in range(UB):
                    prev, cur = k % 2, (k + 1) % 2
                    for uh in range(2):
                        ps = mmp.tile([128, 2, 4, BS], f32, tag="ps")  # [u,(d,g,b)]
                        for d in range(2):
                            for g in range(4):
                                for kc in range(2):
                                    nc.tensor.matmul(
                                        ps[:, d, g, :],
                                        uwsb[:, d, g, uh, kc, :],
                                        h_pp[:, prev, d, kc, :],
                                        start=(kc == 0), stop=(kc == 1))
                        z = work.tile([128, 2, 4, BS], f32, tag=f"z{uh}")
                        nc.vector.tensor_add(z[:], ps[:], zx_sb[:, uh, :, :, k, :])
                        gt = work.tile([128, 2, 4, BS], f32, tag=f"g{uh}")
                        nc.scalar.activation(gt[:, :, 0:3, :], z[:, :, 0:3, :], AF.Sigmoid)
                        nc.scalar.activation(gt[:, :, 3, :], z[:, :, 3, :], AF.Tanh)
                        t1 = work.tile([128, 2, BS], f32, tag=f"t1{uh}")
                        nc.vector.tensor_mul(t1[:], gt[:, :, 1, :], c_sb[:, :, uh, :])
                        t2 = work.tile([128, 2, BS], f32, tag=f"t2{uh}")
                        nc.vector.tensor_mul(t2[:], gt[:, :, 0, :], gt[:, :, 3, :])
                        nc.vector.tensor_add(c_sb[:, :, uh, :], t1[:], t2[:])
                        tcn = work.tile([128, 2, BS], f32, tag=f"tc{uh}")
                        nc.scalar.activation(tcn[:], c_sb[:, :, uh, :], AF.Tanh)
                        nc.vector.tensor_mul(ring[:, :, uh, k, :], gt[:, :, 2, :], tcn[:])
                        nc.scalar.activation(h_pp[:, cur, :, uh, :], ring[:, :, uh, k, :],
                                             AF.Copy)
                if layer == 1:
                    # h1T planes stored in t-order for BOTH dirs:
                    # fwd: steps s0..s0+UB map to t=s0+s; bwd: t = T-1-(s0+s)
                    for uh in range(2):
                        nc.sync.dma_start(hout[0, uh, :, ds(s0, UB), :],
                                          ring[:, 0, uh, :, :])
                        nc.sync.dma_start(hout[1, uh, :, ds(T - UB - s0, UB), :],
                                          ring[:, 1, uh, ::-1, :])
                else:
                    o_r = out.rearrange("b t (dd uh u) -> dd uh t b u", dd=2, uh=2)
                    for d in range(2):
                        if d == 0:
                            tr_in = ring[:, 0, :, :, :]
                        else:
                            # reverse s (-> ascending t) and swap to (uh, b, s)
                            rb = osb.tile([128, 2, BS, UB], f32, tag="rb")
                            nc.vector.tensor_copy(
                                rb[:],
                                ring[:, 1, :, ::-1, :].rearrange("p uh s b -> p uh b s"))
                            tr_in = rb[:]
                        tp = tpp.tile([128, 128], f32, tag="tp")
                        nc.tensor.transpose(tp[:], tr_in, ident[:])
                        ot = osb.tile([128, 128], f32, tag="ot")
                        nc.vector.tensor_copy(ot[:], tp[:])
                        for uh in range(2):
                            src = ot[ts(uh, 64), :]  # partitions (s,b) or (b,s)
                            if d == 0:
                                nc.sync.dma_start(o_r[0, uh][ds(s0, UB), :, :], src)
                            else:
                                nc.sync.dma_start(
                                    o_stg[:, ds(T - UB - s0, UB), ts(uh, 128)], src)

        with ExitStack() as ctx:
            recurrence(1, zx1, u1, h1T, ctx)

        # ---------------- phase C: LN + layer-2 projections ----------------
        with ExitStack() as ctx:
            w2sb = const.tile([128, 2, 4, 2, 4, 128], f32)
            nc.sync.dma_start(w2sb[:], w2.rearrange("d g uh kc k m -> k d g uh kc m"))
            hcp = ctx.enter_context(tc.tile_pool(name="c_hc", bufs=2))
            sqp = ctx.enter_context(tc.tile_pool(name="c_sq", bufs=2))
            hnp = ctx.enter_context(tc.tile_pool(name="c_hn", bufs=2))
            outp = ctx.enter_context(tc.tile_pool(name="c_out", bufs=4))
            smp = ctx.enter_context(tc.tile_pool(name="c_sm", bufs=2))

            for nb in range(NB):
                t0 = nb * 64
                hc = hcp.tile([128, 4, 512], f32, tag="hc")  # chunks [d_src*2+uh]
                for dsrc in range(2):
                    for uh in range(2):
                        cidx = dsrc * 2 + uh
                        nc.sync.dma_start(hc[:, cidx, :],
                                          h1T[dsrc, uh][:, t0:t0 + 64, :])
                sfs = stp.tile([1, 512], f32, tag="s_sum")
                sqs = stp.tile([1, 512], f32, tag="s_sq")
                sq = sqp.tile([128, 4, 512], f32, tag="sq")
                for c in range(4):
                    nc.vector.tensor_mul(sq[:, c, :], hc[:, c, :], hc[:, c, :])
                for c in range(4):
                    nc.tensor.matmul(sfs[:], ones_k[:], hc[:, c, :],
                                     start=(c == 0), stop=(c == 3))
                for c in range(4):
                    nc.tensor.matmul(sqs[:], ones_k[:], sq[:, c, :],
                                     start=(c == 0), stop=(c == 3))
                mu = smp.tile([1, 512], f32, tag="mu")
                nc.scalar.activation(mu[:], sfs[:], AF.Copy, scale=1.0 / 512)
                var = smp.tile([1, 512], f32, tag="var")
                mu2 = smp.tile([1, 512], f32, tag="mu2")
                nc.vector.tensor_mul(mu2[:], mu[:], mu[:])
                nc.scalar.activation(var[:], sqs[:], AF.Copy, scale=1.0 / 512)
                nc.vector.tensor_sub(var[:], var[:], mu2[:])
                sd = smp.tile([1, 512], f32, tag="sd")
                nc.scalar.activation(sd[:], var[:], AF.Sqrt, bias=eps_c[:])
                rs = smp.tile([1, 512], f32, tag="rs")
                nc.vector.reciprocal(rs[:], sd[:])
                mub = mmp.tile([128, 512], f32, tag="ps")
                nc.tensor.matmul(mub[:], ones_m[:], mu[:], start=True, stop=True)
                rsb = mmp.tile([128, 512], f32, tag="ps")
                nc.tensor.matmul(rsb[:], ones_m[:], rs[:], start=True, stop=True)
                hn = hnp.tile([128, 4, 512], f32, tag="hn")
                for c in range(4):
                    nc.vector.tensor_sub(sq[:, c, :], hc[:, c, :], mub[:])
                    nc.vector.tensor_mul(hn[:, c, :], sq[:, c, :], rsb[:])
                for d in range(2):
                    for g in range(4):
                        for uh in range(2):
                            ps = mmp.tile([128, 512], f32, tag="ps")
                            for c in range(4):
                                nc.tensor.matmul(ps[:], w2sb[:, d, g, uh, c, :],
                                                 hn[:, c, :],
                                                 start=(c == 0), stop=(c == 3))
                            ob = outp.tile([128, 512], bf16, tag="ob")
                            eng = nc.vector.tensor_copy if (g % 2) else (
                                lambda o, i: nc.scalar.activation(o, i, AF.Copy))
                            eng(ob[:], ps[:])
                            plane = zx2[d, uh, g]
                            if d == 0:
                                nc.sync.dma_start(plane[:, t0:t0 + 64, :], ob[:])
                            else:
                                dst = bass.AP(
                                    tensor=plane.tensor,
                                    offset=plane.offset + ((T - 1 - t0) * BS),
                                    ap=[[T * BS, 128], [-BS, 64], [1, BS]],
                                )
                                nc.sync.dma_start(dst, ob[:])

        with ExitStack() as ctx:
            recurrence(2, zx2, u2, None, ctx)

        # phase E: copy staged bwd half into the output (static)
        nc.sync.dma_start(out[:, :, U:2 * U], o_stg[:])

        if dbg:
            dzx1 = nc.dram_tensor("dbg_zx1", [2, 2, 4, 128, T, BS], bf16,
                                  kind="ExternalOutput").ap()
            dh1 = nc.dram_tensor("dbg_h1T", [2, 2, 128, T, BS], f32,
                                 kind="ExternalOutput").ap()
            dzx2 = nc.dram_tensor("dbg_zx2", [2, 2, 4, 128, T, BS], bf16,
                                  kind="ExternalOutput").ap()
            for d in range(2):
                for uh in range(2):
                    nc.sync.dma_start(dh1[d, uh], h1T[d, uh])
                    for g in range(4):
                        nc.sync.dma_start(dzx1[d, uh, g], zx1[d, uh, g])
                        nc.sync.dma_start(dzx2[d, uh, g], zx2[d, uh, g])

    if split:
        split_ctrl_waits(nc)
    return nc


# ---------------------------------------------------------------- host packing
def _pack_w1(Wf, Wb):
    w = np.zeros((2, 4, 2, F, 128), np.float32)
    for d, Wd in enumerate((Wf, Wb)):
        for g in range(4):
            og = GMAP[g]
            for uh in range(2):
                w[d, g, uh] = Wd[:, og * U + uh * 128: og * U + (uh + 1) * 128]
    return w.astype(ml_bf16)


def _pack_u(Uf, Ub):
    u = np.zeros((2, 4, 2, 2, 128, 128), np.float32)
    for d, Ud in enumerate((Uf, Ub)):
        for g in range(4):
            og = GMAP[g]
            for uh in range(2):
                for kc in range(2):
                    u[d, g, uh, kc] = Ud[kc * 128:(kc + 1) * 128,
                                         og * U + uh * 128: og * U + (uh + 1) * 128]
    return u.astype(ml_bf16)


def _pack_w2(W2f, W2b, gamma):
    w = np.zeros((2, 4, 2, 4, 128, 128), np.float32)
    for d, Wd in enumerate((W2f, W2b)):
        Wg = gamma[:, None] * Wd
        for g in range(4):
            og = GMAP[g]
            for uh in range(2):
                for kc in range(4):
                    w[d, g, uh, kc] = Wg[kc * 128:(kc + 1) * 128,
                                         og * U + uh * 128: og * U + (uh + 1) * 128]
    return w


_CACHE = {}


def kernel(x, W1f, U1f, b1f, W1b, U1b, b1b, gamma, beta,
           W2f, U2f, b2f, W2b, U2b, b2b, _T=None, _dbg=False):
    T = _T or x.shape[1]
    assert np.abs(b1f).max() == 0 and np.abs(b1b).max() == 0, "bias folding not implemented"
    assert np.abs(b2f).max() == 0 and np.abs(b2b).max() == 0
    assert np.abs(beta).max() == 0, "beta folding not implemented"

    key = (T, _dbg)
    if key not in _CACHE:
        _CACHE[key] = build_program(T, dbg=_dbg)
    nc = _CACHE[key]

    w1 = _pack_w1(np.asarray(W1f), np.asarray(W1b))
    u1 = _pack_u(np.asarray(U1f), np.asarray(U1b))
    w2 = _pack_w2(np.asarray(W2f), np.asarray(W2b), np.asarray(gamma))
    u2 = _pack_u(np.asarray(U2f), np.asarray(U2b))

    x = np.asarray(x)
    in_maps = []
    for c in range(NCORES):
        in_maps.append({
            "x_sh": np.ascontiguousarray(x[c * BS:(c + 1) * BS, :T]),
            "w1": w1, "u1": u1, "w2": w2, "u2": u2,
        })
    res = bass_utils.run_bass_kernel_spmd(nc, in_maps, core_ids=list(range(NCORES)))
    global LAST_RESULT
    LAST_RESULT = res
    out = np.concatenate([res.results[c]["out_sh"] for c in range(NCORES)], axis=0)
    return out


LAST_RESULT = None


# revision 19
# speedup vs baseline: 1.2677x; 1.2677x over previous
"""BiLSTM 2-layer + LayerNorm Trainium2 kernel.

Strategy: data-parallel over batch (8 cores x 8 batch). Each core runs the
full network on its batch shard:
  A: input projections zx1 = x @ W1{f,b}   (bwd written time-reversed)
  B: layer-1 fwd+bwd recurrences, interleaved per step
  C: LayerNorm (folded into projection) + zx2 = LN(h1) @ W2{f,b}
  D: layer-2 recurrences, transposed output written directly

Layouts (per core):
  zx (DRAM, bf16): [d2, uh2, g4, u128, T, b8]; gate order [i, f, o, g~];
    bwd (d=1) plane stored time-reversed, so both recurrences read ascending.
  h1T (DRAM, f32): [d2, uh2, u128, T, b8]; bwd plane in step order (rev time).
  Recurrence state: h.T [u128, kc2, b8] (bf16 for matmul), c [u128, d2, uh2, b8] f32.
  Recurrent GEMM: z.T[(g,uh) tile, (d,b)] += U[kc-chunk, (g,uh)-cols].T @ h.T[kc]
"""
import contextlib
import ctypes
import sys
import types

import ml_dtypes
import numpy as np

import concourse.bass as bass
import concourse.tile as tile
from concourse import mybir
from concourse import bass_utils
from concourse.bass import ds, ts
from concourse.masks import make_identity
from concourse.vector_clock import ScopedClock
from contextlib import ExitStack

# ---------------------------------------------------------------- boot patches
MAXW = 1  # this walrus build allows only 1 sem-wait per instruction


def _patched_drain_and_barrier(self, tick_clock, wait_clock):
    drain_inst = self.nc.sync.drain()
    wait_clock.add_sem_waits(drain_inst.ins, ScopedClock({None: tick_clock.global_clock}))
    si = drain_inst.ins.sync_info
    waits = list(si.on_wait) if si is not None and si.on_wait else []
    if len(waits) > MAXW:
        si.on_wait = waits[:MAXW]
        rest = waits[MAXW:]
        while rest:
            d2 = self.nc.sync.drain()
            d2.ins.sync_info = mybir.SyncInfo(on_wait=rest[:MAXW], on_update=[])
            rest = rest[MAXW:]
    self.nc.all_engine_barrier()
    assert self.sems is not None
    popped = self.nc._tile_sem_poison_stack.pop()
    assert popped is self._sem_poison
    self.nc.clear_and_free_semaphores(list(self.sems.allocated().values()))
    self.nc.all_engine_barrier()


tile.TileContext._drain_and_barrier = _patched_drain_and_barrier


def split_ctrl_waits(nc):
    """Hoist extra sem-waits (>1 per instruction) onto preceding NoOps."""
    n_split = 0
    for f in nc.m.functions:
        for bb in f.blocks:
            new_insts = []
            for inst in bb.instructions:
                si = getattr(inst, "sync_info", None)
                waits = list(si.on_wait) if si is not None and si.on_wait else []
                if len(waits) > MAXW:
                    rest, tail = waits[:-MAXW], waits[-MAXW:]
                    while rest:
                        d = mybir.InstNoOp(
                            name=nc.get_next_instruction_name(),
                            engine=inst.engine,
                            bass_nofuse=True,
                            sync_info=mybir.SyncInfo(on_wait=rest[:MAXW], on_update=[]),
                        )
                        new_insts.append(d)
                        rest = rest[MAXW:]
                    si.on_wait = tail
                    n_split += 1
                new_insts.append(inst)
            bb.instructions[:] = new_insts
    return n_split


# ---------------------------------------------------------------- constants
B, T_FULL, F, U = 64, 1024, 128, 256
NCORES = 8
BS = B // NCORES          # batch per core
LN_EPS = 1e-3
UB = 32                   # recurrence steps per For_i iteration

f32 = mybir.dt.float32
bf16 = mybir.dt.bfloat16
AF = mybir.ActivationFunctionType

# gate order in our layout: [i, f, o, g~]; original keras order [i, f, g, o]
GMAP = [0, 1, 3, 2]  # ours g -> original gate index
ml_bf16 = ml_dtypes.bfloat16


# ---------------------------------------------------------------- program
def build_program(T=T_FULL, dbg=False, split=True):
    nc = bass.Bass("TRN2", target_bir_lowering=False, debug=False)

    x_in = nc.dram_tensor("x_sh", [BS, T, F], f32, kind="ExternalInput").ap()
    # packed weights, host-prepared (see pack_weights):
    w1 = nc.dram_tensor("w1", [2, 4, 2, F, 128], bf16, kind="ExternalInput").ap()
    u1 = nc.dram_tensor("u1", [2, 4, 2, 2, 128, 128], bf16, kind="ExternalInput").ap()
    w2 = nc.dram_tensor("w2", [2, 4, 2, 4, 128, 128], f32, kind="ExternalInput").ap()
    u2 = nc.dram_tensor("u2", [2, 4, 2, 2, 128, 128], bf16, kind="ExternalInput").ap()
    out = nc.dram_tensor("out_sh", [BS, T, 2 * U], f32, kind="ExternalOutput").ap()

    assert T % 64 == 0, "phase A/C block size needs T % 64 == 0"
    NB = T // 64  # number of 512-col (t64 x b8) blocks

    with tile.TileContext(nc) as tc, ExitStack() as octx:
        const = octx.enter_context(tc.tile_pool(name="const", bufs=1))
        dram = octx.enter_context(tc.tile_pool(name="dram", bufs=1, space="DRAM"))
        # PSUM is bump-allocated across the whole context: share pools globally.
        # Budget (8 banks): mmp 4 (tag ps) + tpp 2 (tag tp) + stp 2 (s_sum/s_sq)
        mmp = octx.enter_context(tc.tile_pool(name="ps_mm", bufs=4, space="PSUM"))
        tpp = octx.enter_context(tc.tile_pool(name="ps_tp", bufs=2, space="PSUM"))
        stp = octx.enter_context(tc.tile_pool(name="ps_st", bufs=1, space="PSUM"))

        zx1 = dram.tile([2, 2, 4, 128, T, BS], bf16)
        h1T = dram.tile([2, 2, 128, T, BS], f32)
        zx2 = dram.tile([2, 2, 4, 128, T, BS], bf16)
        o_stg = dram.tile([BS, T, 256], f32)  # bwd half of out (dynamic rev writes)

        ident = const.tile([128, 128], f32)
        make_identity(nc, ident)
        ones_k = const.tile([128, 1], f32)
        nc.vector.memset(ones_k, 1.0)
        ones_m = const.tile([1, 128], f32)
        nc.vector.memset(ones_m, 1.0)
        eps_c = const.tile([1, 1], f32)
        nc.vector.memset(eps_c, LN_EPS)

        # ---------------- phase A: layer-1 projections ----------------
        with ExitStack() as ctx:
            w1sb = const.tile([F, 2, 4, 2, 128], bf16)
            nc.sync.dma_start(w1sb[:], w1.rearrange("d g uh f m -> f d g uh m"))
            stage = ctx.enter_context(tc.tile_pool(name="a_stage", bufs=3))
            xtp = ctx.enter_context(tc.tile_pool(name="a_xt", bufs=2))
            outp = ctx.enter_context(tc.tile_pool(name="a_out", bufs=4))

            for nb in range(NB):
                t0 = nb * 64
                xT = xtp.tile([128, 512], bf16, tag="xT")  # [f, (t64 b8)]
                for j in range(4):
                    xa = stage.tile([128, F], f32, tag="xa")  # [(t16 b8), f]
                    src = bass.AP(
                        tensor=x_in.tensor,
                        offset=x_in.offset + (t0 + j * 16) * F,
                        ap=[[F, 16], [T * F, BS], [1, F]],
                    )
                    nc.sync.dma_start(xa[:], src)
                    tp = tpp.tile([128, 128], f32, tag="tp")
                    nc.tensor.transpose(tp[:], xa[:], ident[:])
                    nc.vector.tensor_copy(xT[:, ts(j, 128)], tp[:])
                for d in range(2):
                    for g in range(4):
                        for uh in range(2):
                            ps = mmp.tile([128, 512], f32, tag="ps")
                            nc.tensor.matmul(ps[:], w1sb[:, d, g, uh, :], xT[:],
                                             start=True, stop=True)
                            ob = outp.tile([128, 512], bf16, tag="ob")
                            eng = nc.vector.tensor_copy if (g % 2) else (
                                lambda o, i: nc.scalar.activation(o, i, AF.Copy))
                            eng(ob[:], ps[:])
                            plane = zx1[d, uh, g]  # [128, T, BS]
                            if d == 0:
                                nc.sync.dma_start(plane[:, t0:t0 + 64, :], ob[:])
                            else:
                                dst = bass.AP(
                                    tensor=plane.tensor,
                                    offset=plane.offset + ((T - 1 - t0) * BS),
                                    ap=[[T * BS, 128], [-BS, 64], [1, BS]],
                                )
                                nc.sync.dma_start(dst, ob[:])

        # ---------------- phase B/D: recurrences ----------------
        def recurrence(layer, zx, u_w, hout, ctx):
            """layer 1: write h1T planes; layer 2: write transposed out."""
            uwsb = const.tile([128, 2, 4, 2, 2, 128], bf16, tag=f"uw{layer}")
            nc.sync.dma_start(uwsb[:], u_w.rearrange("d g uh kc k m -> k d g uh kc m"))

            state = ctx.enter_context(tc.tile_pool(name=f"r{layer}_state", bufs=1))
            zxp = ctx.enter_context(tc.tile_pool(name=f"r{layer}_zx", bufs=2))
            work = ctx.enter_context(tc.tile_pool(name=f"r{layer}_work", bufs=3))
            ringp = ctx.enter_context(tc.tile_pool(name=f"r{layer}_ring", bufs=2))
            osb = ctx.enter_context(tc.tile_pool(name=f"r{layer}_osb", bufs=2))

            # h ping-pong (bf16, matmul rhs layout [u, kc, b]); c state f32
            h_pp = state.tile([128, 2, 2, 2, BS], bf16)  # [u, pp, d, kc, b]
            c_sb = state.tile([128, 2, 2, BS], f32)      # [u, d, uh, b]
            nc.vector.memset(h_pp[:], 0.0)
            nc.vector.memset(c_sb[:], 0.0)

            with tc.For_i(0, T, UB) as s0:
                zx_sb = zxp.tile([128, 2, 2, 4, UB, BS], bf16, tag="zx_sb")  # [u, uh, d, g, s, b]
                for d in range(2):
                    for uh in range(2):
                        nc.scalar.dma_start(
                            zx_sb[:, uh, d, :, :, :],
                            zx[d, uh, :, :, ds(s0, UB), :].rearrange("g u s b -> u g s b"))
                ring = ringp.tile([128, 2, 2, UB, BS], f32, tag="ring")  # [u, d, uh, s, b]
                for k in range(UB):
                    prev, cur = k % 2, (k + 1) % 2
                    for uh in range(2):
                        ps = mmp.tile([128, 2, 4, BS], f32, tag="ps")  # [u,(d,g,b)]
                        for d in range(2):
                            for g in range(4):
                                for kc in range(2):
                                    nc.tensor.matmul(
                                        ps[:, d, g, :],
                                        uwsb[:, d, g, uh, kc, :],
                                        h_pp[:, prev, d, kc, :],
                                        start=(kc == 0), stop=(kc == 1))
                        z = work.tile([128, 2, 4, BS], f32, tag=f"z{uh}")
                        nc.vector.tensor_add(z[:], ps[:], zx_sb[:, uh, :, :, k, :])
                        gt = work.tile([128, 2, 4, BS], f32, tag=f"g{uh}")
                        nc.scalar.activation(gt[:, :, 0:3, :], z[:, :, 0:3, :], AF.Sigmoid)
                        nc.scalar.activation(gt[:, :, 3, :], z[:, :, 3, :], AF.Tanh)
                        t1 = work.tile([128, 2, BS], f32, tag=f"t1{uh}")
                        nc.vector.tensor_mul(t1[:], gt[:, :, 1, :], c_sb[:, :, uh, :])
                        t2 = work.tile([128, 2, BS], f32, tag=f"t2{uh}")
                        nc.vector.tensor_mul(t2[:], gt[:, :, 0, :], gt[:, :, 3, :])
                        nc.vector.tensor_add(c_sb[:, :, uh, :], t1[:], t2[:])
                        tcn = work.tile([128, 2, BS], f32, tag=f"tc{uh}")
                        nc.scalar.activation(tcn[:], c_sb[:, :, uh, :], AF.Tanh)
                        nc.vector.tensor_mul(ring[:, :, uh, k, :], gt[:, :, 2, :], tcn[:])
                        nc.scalar.activation(h_pp[:, cur, :, uh, :], ring[:, :, uh, k, :],
                                             AF.Copy)
                if layer == 1:
                    # h1T planes stored in t-order for BOTH dirs:
                    # fwd: steps s0..s0+UB map to t=s0+s; bwd: t = T-1-(s0+s)
                    for uh in range(2):
                        nc.sync.dma_start(hout[0, uh, :, ds(s0, UB), :],
                                          ring[:, 0, uh, :, :])
                        nc.sync.dma_start(hout[1, uh, :, ds(T - UB - s0, UB), :],
                                          ring[:, 1, uh, ::-1, :])
                else:
                    o_r = out.rearrange("b t (dd uh u) -> dd uh t b u", dd=2, uh=2)
                    for d in range(2):
                        for uh in range(2):
                            for j in range(UB // 16):
                                blk = ring[:, d, uh, j * 16:(j + 1) * 16, :]
                                if d == 0:
                                    tr_in = blk  # [128, (s16, b8)]
                                else:
                                    # reverse s (-> ascending t), swap to (b, s)
                                    rb = osb.tile([128, BS, 16], f32, tag="rb")
                                    nc.vector.tensor_copy(
                                        rb[:],
                                        blk[:, ::-1, :].rearrange("p s b -> p b s"))
                                    tr_in = rb[:]
                                tp = tpp.tile([128, 128], f32, tag="tp")
                                nc.tensor.transpose(tp[:], tr_in, ident[:])
                                ot = osb.tile([128, 128], f32, tag="ot")
                                nc.vector.tensor_copy(ot[:], tp[:])
                                if d == 0:
                                    nc.sync.dma_start(
                                        o_r[0, uh][ds(s0 + 16 * j, 16), :, :], ot[:])
                                else:
                                    nc.sync.dma_start(
                                        o_stg[:, ds(T - 16 * (j + 1) - s0, 16),
                                              ts(uh, 128)], ot[:])

        with ExitStack() as ctx:
            recurrence(1, zx1, u1, h1T, ctx)

        # ---------------- phase C: LN + layer-2 projections ----------------
        with ExitStack() as ctx:
            w2sb = const.tile([128, 2, 4, 2, 4, 128], f32)
            nc.sync.dma_start(w2sb[:], w2.rearrange("d g uh kc k m -> k d g uh kc m"))
            hcp = ctx.enter_context(tc.tile_pool(name="c_hc", bufs=2))
            sqp = ctx.enter_context(tc.tile_pool(name="c_sq", bufs=2))
            hnp = ctx.enter_context(tc.tile_pool(name="c_hn", bufs=2))
            outp = ctx.enter_context(tc.tile_pool(name="c_out", bufs=4))
            smp = ctx.enter_context(tc.tile_pool(name="c_sm", bufs=2))

            for nb in range(NB):
                t0 = nb * 64
                hc = hcp.tile([128, 4, 512], f32, tag="hc")  # chunks [d_src*2+uh]
                for dsrc in range(2):
                    for uh in range(2):
                        cidx = dsrc * 2 + uh
                        nc.sync.dma_start(hc[:, cidx, :],
                                          h1T[dsrc, uh][:, t0:t0 + 64, :])
                sfs = stp.tile([1, 512], f32, tag="s_sum")
                sqs = stp.tile([1, 512], f32, tag="s_sq")
                sq = sqp.tile([128, 4, 512], f32, tag="sq")
                for c in range(4):
                    nc.vector.tensor_mul(sq[:, c, :], hc[:, c, :], hc[:, c, :])
                for c in range(4):
                    nc.tensor.matmul(sfs[:], ones_k[:], hc[:, c, :],
                                     start=(c == 0), stop=(c == 3))
                for c in range(4):
                    nc.tensor.matmul(sqs[:], ones_k[:], sq[:, c, :],
                                     start=(c == 0), stop=(c == 3))
                mu = smp.tile([1, 512], f32, tag="mu")
                nc.scalar.activation(mu[:], sfs[:], AF.Copy, scale=1.0 / 512)
                var = smp.tile([1, 512], f32, tag="var")
                mu2 = smp.tile([1, 512], f32, tag="mu2")
                nc.vector.tensor_mul(mu2[:], mu[:], mu[:])
                nc.scalar.activation(var[:], sqs[:], AF.Copy, scale=1.0 / 512)
                nc.vector.tensor_sub(var[:], var[:], mu2[:])
                sd = smp.tile([1, 512], f32, tag="sd")
                nc.scalar.activation(sd[:], var[:], AF.Sqrt, bias=eps_c[:])
                rs = smp.tile([1, 512], f32, tag="rs")
                nc.vector.reciprocal(rs[:], sd[:])
                mub = mmp.tile([128, 512], f32, tag="ps")
                nc.tensor.matmul(mub[:], ones_m[:], mu[:], start=True, stop=True)
                rsb = mmp.tile([128, 512], f32, tag="ps")
                nc.tensor.matmul(rsb[:], ones_m[:], rs[:], start=True, stop=True)
                hn = hnp.tile([128, 4, 512], f32, tag="hn")
                for c in range(4):
                    nc.vector.tensor_sub(sq[:, c, :], hc[:, c, :], mub[:])
                    nc.vector.tensor_mul(hn[:, c, :], sq[:, c, :], rsb[:])
                for d in range(2):
                    for g in range(4):
                        for uh in range(2):
                            ps = mmp.tile([128, 512], f32, tag="ps")
                            for c in range(4):
                                nc.tensor.matmul(ps[:], w2sb[:, d, g, uh, c, :],
                                                 hn[:, c, :],
                                                 start=(c == 0), stop=(c == 3))
                            ob = outp.tile([128, 512], bf16, tag="ob")
                            eng = nc.vector.tensor_copy if (g % 2) else (
                                lambda o, i: nc.scalar.activation(o, i, AF.Copy))
                            eng(ob[:], ps[:])
                            plane = zx2[d, uh, g]
                            if d == 0:
                                nc.sync.dma_start(plane[:, t0:t0 + 64, :], ob[:])
                            else:
                                dst = bass.AP(
                                    tensor=plane.tensor,
                                    offset=plane.offset + ((T - 1 - t0) * BS),
                                    ap=[[T * BS, 128], [-BS, 64], [1, BS]],
                                )
                                nc.sync.dma_start(dst, ob[:])

        with ExitStack() as ctx:
            recurrence(2, zx2, u2, None, ctx)

        # phase E: copy staged bwd half into the output (static)
        nc.sync.dma_start(out[:, :, U:2 * U], o_stg[:])

        if dbg:
            dzx1 = nc.dram_tensor("dbg_zx1", [2, 2, 4, 128, T, BS], bf16,
                                  kind="ExternalOutput").ap()
            dh1 = nc.dram_tensor("dbg_h1T", [2, 2, 128, T, BS], f32,
                                 kind="ExternalOutput").ap()
            dzx2 = nc.dram_tensor("dbg_zx2", [2, 2, 4, 128, T, BS], bf16,
                                  kind="ExternalOutput").ap()
            for d in range(2):
                for uh in range(2):
                    nc.sync.dma_start(dh1[d, uh], h1T[d, uh])
                    for g in range(4):
                        nc.sync.dma_start(dzx1[d, uh, g], zx1[d, uh, g])
                        nc.sync.dma_start(dzx2[d, uh, g], zx2[d, uh, g])

    if split:
        split_ctrl_waits(nc)
    return nc


# ---------------------------------------------------------------- host packing
def _pack_w1(Wf, Wb):
    w = np.zeros((2, 4, 2, F, 128), np.float32)
    for d, Wd in enumerate((Wf, Wb)):
        for g in range(4):
            og = GMAP[g]
            for uh in range(2):
                w[d, g, uh] = Wd[:, og * U + uh * 128: og * U + (uh + 1) * 128]
    return w.astype(ml_bf16)


def _pack_u(Uf, Ub):
    u = np.zeros((2, 4, 2, 2, 128, 128), np.float32)
    for d, Ud in enumerate((Uf, Ub)):
        for g in range(4):
            og = GMAP[g]
            for uh in range(2):
                for kc in range(2):
                    u[d, g, uh, kc] = Ud[kc * 128:(kc + 1) * 128,
                                         og * U + uh * 128: og * U + (uh + 1) * 128]
    return u.astype(ml_bf16)


def _pack_w2(W2f, W2b, gamma):
    w = np.zeros((2, 4, 2, 4, 128, 128), np.float32)
    for d, Wd in enumerate((W2f, W2b)):
        Wg = gamma[:, None] * Wd
        for g in range(4):
            og = GMAP[g]
            for uh in range(2):
                for kc in range(4):
                    w[d, g, uh, kc] = Wg[kc * 128:(kc + 1) * 128,
                                         og * U + uh * 128: og * U + (uh + 1) * 128]
    return w


_CACHE = {}


def kernel(x, W1f, U1f, b1f, W1b, U1b, b1b, gamma, beta,
           W2f, U2f, b2f, W2b, U2b, b2b, _T=None, _dbg=False):
    T = _T or x.shape[1]
    assert np.abs(b1f).max() == 0 and np.abs(b1b).max() == 0, "bias folding not implemented"
    assert np.abs(b2f).max() == 0 and np.abs(b2b).max() == 0
    assert np.abs(beta).max() == 0, "beta folding not implemented"

    key = (T, _dbg)
    if key not in _CACHE:
        _CACHE[key] = build_program(T, dbg=_dbg)
    nc = _CACHE[key]

    w1 = _pack_w1(np.asarray(W1f), np.asarray(W1b))
    u1 = _pack_u(np.asarray(U1f), np.asarray(U1b))
    w2 = _pack_w2(np.asarray(W2f), np.asarray(W2b), np.asarray(gamma))
    u2 = _pack_u(np.asarray(U2f), np.asarray(U2b))

    x = np.asarray(x)
    in_maps = []
    for c in range(NCORES):
        in_maps.append({
            "x_sh": np.ascontiguousarray(x[c * BS:(c + 1) * BS, :T]),
            "w1": w1, "u1": u1, "w2": w2, "u2": u2,
        })
    res = bass_utils.run_bass_kernel_spmd(nc, in_maps, core_ids=list(range(NCORES)))
    global LAST_RESULT
    LAST_RESULT = res
    out = np.concatenate([res.results[c]["out_sh"] for c in range(NCORES)], axis=0)
    return out


LAST_RESULT = None
